# revision 1
# baseline (speedup 1.0000x reference)
"""MoE kernel for TRN2, 8 NeuronCores, data-parallel over the batch dim.

Reference computation (B=8192, D=1024, H=1024, E=16):
    weights = softmax(x @ Wg + bg, axis=1)            # [B, E]
    h       = relu(einsum('bd,edh->beh', x, W1) + b1) # [B, E, H]
    eo      = einsum('beh,eh->be', h, W2) + b2        # [B, E]
    out     = sum(eo * weights, axis=1, keepdims=True)# [B, 1]

Strategy:
  - Shard B over 8 cores (1024 rows/core); weights replicated.
  - All matmuls contract over the partition dim, so x is transposed on the
    HOST (free) and each core gets xT [D, 1024] resident in SBUF.
  - Stage 1 (per (e, h_tile)=t of 128): psum[h=128, b=512x2] accumulated
    over 8 d-tiles; float32r matmuls (full PE rate at N>=256).
  - ReLU+b1 via ScalarE activation (bias is per-partition in h-major layout).
  - Stage 2: W2 built block-diagonal on host -> every t contributes one
    [128hx16e] @ [128h, 512b] matmul accumulating into ONE [16, 1024] psum
    tile; all 16 experts' outputs land stacked on partitions 0..15.
  - Gating: fp32 matmuls into [128b, 16e] psum; softmax along the free dim;
    exp(bg) folded in multiplicatively (softmax is shift/scale invariant).
  - Combine: eoT + b2 -> PE-transpose 16x128 chunks -> [128b, 16e]; multiply
    by gate weights, reduce along free dim -> [128, 1] -> DMA out.
"""

import numpy as np

import concourse.bacc as bacc
import concourse.bass as bass
import concourse.mybir as mybir
from concourse import tile
from concourse.bass_utils import run_bass_kernel_spmd

B, D, H, E = 8192, 1024, 1024, 16
N_CORES = 8
BS = B // N_CORES  # 1024 batch rows per core
NB = BS // 128     # 8 b-tiles of 128
BH = 512           # half-batch moving-operand width (one psum bank)
DT = D // 128      # 8 d-tiles
HT = H // 128      # 8 h-tiles
T = E * HT         # 128 (e, h_tile) pairs

F32 = mybir.dt.float32
F32R = mybir.dt.float32r
AF = mybir.ActivationFunctionType
AX = mybir.AxisListType


def build_bass():
    nc = bacc.Bacc("TRN2", target_bir_lowering=False, debug=False)
    xt_d = nc.dram_tensor("xt", [D, BS], F32R, kind="ExternalInput")
    w1_d = nc.dram_tensor("w1p", [T, 128, DT * 128], F32R, kind="ExternalInput")
    b1t_d = nc.dram_tensor("b1t", [128, T], F32, kind="ExternalInput")
    w2bd_d = nc.dram_tensor("w2bd", [128, T * E], F32R, kind="ExternalInput")
    wgp_d = nc.dram_tensor("wgp", [128, DT * E], F32R, kind="ExternalInput")
    ebg_d = nc.dram_tensor("ebg", [128, E], F32, kind="ExternalInput")
    b2_d = nc.dram_tensor("b2p", [E, 1], F32, kind="ExternalInput")
    id16_d = nc.dram_tensor("id16", [E, E], F32, kind="ExternalInput")
    y_d = nc.dram_tensor("y", [BS, 1], F32, kind="ExternalOutput")

    with tile.TileContext(nc) as tc:
        with (
            tc.tile_pool(name="const", bufs=1) as cpool,
            tc.tile_pool(name="w1", bufs=4) as w1pool,
            tc.tile_pool(name="hrelu", bufs=4) as hpool,
            tc.tile_pool(name="sm", bufs=2) as smpool,
            tc.tile_pool(name="ps_h", bufs=2, space=bass.MemorySpace.PSUM) as psh,
            tc.tile_pool(name="ps_eo", bufs=1, space=bass.MemorySpace.PSUM) as pseo,
            tc.tile_pool(name="ps_s", bufs=2, space=bass.MemorySpace.PSUM) as pss,
        ):
            # ---- resident tensors ----
            xt_sb = []
            for d in range(DT):
                tl = cpool.tile([128, BS], F32R, tag=f"xt{d}")
                nc.sync.dma_start(tl[:], xt_d[d * 128:(d + 1) * 128, :])
                xt_sb.append(tl)
            w2bd_sb = cpool.tile([128, T * E], F32R, tag="w2bd")
            nc.sync.dma_start(w2bd_sb[:], w2bd_d[:])
            b1t_sb = cpool.tile([128, T], F32, tag="b1t")
            nc.sync.dma_start(b1t_sb[:], b1t_d[:])
            wgp_sb = cpool.tile([128, DT * E], F32R, tag="wgp")
            nc.sync.dma_start(wgp_sb[:], wgp_d[:])
            ebg_sb = cpool.tile([128, E], F32, tag="ebg")
            nc.sync.dma_start(ebg_sb[:], ebg_d[:])
            b2_sb = cpool.tile([E, 1], F32, tag="b2")
            nc.sync.dma_start(b2_sb[:], b2_d[:])
            id16_sb = cpool.tile([E, E], F32, tag="id16")
            nc.sync.dma_start(id16_sb[:], id16_d[:])
            w_all = cpool.tile([128, NB, E], F32, tag="wall")  # gate weights
            eo_sb = cpool.tile([E, BS], F32, tag="eo")         # expert outs ^T

            # ---- gating: logits -> softmax along free dim ----
            for bt in range(NB):
                ps_g = pss.tile([128, E], F32, tag="sps")
                for d in range(DT):
                    nc.tensor.matmul(
                        ps_g[:],
                        xt_sb[d][:, bt * 128:(bt + 1) * 128],
                        wgp_sb[:, d * E:(d + 1) * E],
                        start=(d == 0), stop=(d == DT - 1),
                        skip_group_check=True,
                    )
                pexp = smpool.tile([128, E], F32, tag="pexp")
                nc.scalar.activation(pexp[:], ps_g[:], AF.Exp)
                nc.vector.tensor_mul(pexp[:], pexp[:], ebg_sb[:])
                ssum = smpool.tile([128, 1], F32, tag="ssum")
                nc.vector.reduce_sum(ssum[:], pexp[:], axis=AX.X)
                rsum = smpool.tile([128, 1], F32, tag="rsum")
                nc.vector.reciprocal(rsum[:], ssum[:])
                nc.vector.tensor_scalar_mul(w_all[:, bt, :], pexp[:], rsum[:])

            # ---- main loop over t=(e, h_tile) ----
            eo_ps = pseo.tile([E, BS], F32)

            def emit_stage2(t, hr):
                for bh in range(2):
                    nc.tensor.matmul(
                        eo_ps[:, bh * BH:(bh + 1) * BH],
                        w2bd_sb[:, t * E:(t + 1) * E],
                        hr[:, bh * BH:(bh + 1) * BH],
                        start=(t == 0), stop=(t == T - 1),
                        skip_group_check=True,
                    )

            pending = []
            for t in range(T):
                w1t = w1pool.tile([128, DT * 128], F32R, tag="w1t")
                nc.sync.dma_start(w1t[:], w1_d[t, :, :])
                ps1 = psh.tile([128, BS], F32, tag="ps1")
                for d in range(DT):
                    lhs = w1t[:, d * 128:(d + 1) * 128]
                    for bh in range(2):
                        nc.tensor.matmul(
                            ps1[:, bh * BH:(bh + 1) * BH],
                            lhs,
                            xt_sb[d][:, bh * BH:(bh + 1) * BH],
                            start=(d == 0), stop=(d == DT - 1),
                            skip_group_check=True,
                        )
                if pending:
                    emit_stage2(*pending.pop())
                hr = hpool.tile([128, BS], F32R, tag="hr")
                for bh in range(2):
                    nc.scalar.activation(
                        hr[:, bh * BH:(bh + 1) * BH],
                        ps1[:, bh * BH:(bh + 1) * BH],
                        AF.Relu,
                        bias=b1t_sb[:, t:t + 1],
                    )
                pending.append((t, hr))
            emit_stage2(*pending.pop())

            # ---- combine: (eoT + b2) -> transpose -> * gates -> reduce ----
            nc.vector.tensor_scalar_add(eo_sb[:], eo_ps[:], b2_sb[:])
            for bt in range(NB):
                tps = pss.tile([128, E], F32, tag="sps")
                nc.tensor.transpose(
                    tps[:], eo_sb[:, bt * 128:(bt + 1) * 128], id16_sb[:]
                )
                eo_bt = smpool.tile([128, E], F32, tag="eobt")
                nc.vector.tensor_copy(eo_bt[:], tps[:])
                prod = smpool.tile([128, E], F32, tag="prod")
                nc.vector.tensor_mul(prod[:], eo_bt[:], w_all[:, bt, :])
                y_t = smpool.tile([128, 1], F32, tag="yt")
                nc.vector.reduce_sum(y_t[:], prod[:], axis=AX.X)
                nc.sync.dma_start(y_d[bt * 128:(bt + 1) * 128, :], y_t[:])
    nc.compile()
    return nc


def round_fp32r(a):
    """Round fp32 to the FP32R format: 11-bit mantissa, RNE, low 12 bits 0."""
    u = np.ascontiguousarray(a, dtype=np.float32).view(np.uint32)
    lsb = (u >> np.uint32(12)) & np.uint32(1)
    r = (u + np.uint32(0x7FF) + lsb) & np.uint32(0xFFFFF000)
    return r.view(np.float32)


def prep_inputs(x, W1, b1, W2, b2, Wg, bg):
    """Host-side data prep. Returns (shared_map, per_core_xt)."""
    f = np.float32
    # W1 [E, D, H] -> [t=(e,ht), d_in, (d_t, h_in)] so each t is one
    # contiguous 512KB block whose SBUF layout is [128 d_in, 8 d_t * 128 h]
    w1p = np.ascontiguousarray(
        W1.reshape(E, DT, 128, HT, 128).transpose(0, 3, 2, 1, 4)
        .reshape(T, 128, DT * 128).astype(f))
    w1p = round_fp32r(w1p)
    b1t = np.ascontiguousarray(
        b1.reshape(E, HT, 128).transpose(2, 0, 1).reshape(128, T).astype(f))
    w2bd = np.zeros((128, T, E), dtype=f)
    for t in range(T):
        e, ht = divmod(t, HT)
        w2bd[:, t, e] = W2[e, ht * 128:(ht + 1) * 128]
    w2bd = round_fp32r(w2bd.reshape(128, T * E))
    wgp = np.ascontiguousarray(
        Wg.reshape(DT, 128, E).transpose(1, 0, 2).reshape(128, DT * E).astype(f))
    wgp = round_fp32r(wgp)
    ebg = np.broadcast_to(np.exp(bg.astype(f))[None, :], (128, E)).copy()
    b2p = np.ascontiguousarray(b2.astype(f).reshape(E, 1))
    id16 = np.eye(E, dtype=f)
    shared = {"w1p": w1p, "b1t": b1t, "w2bd": w2bd, "wgp": wgp,
              "ebg": ebg, "b2p": b2p, "id16": id16}
    xT = round_fp32r(np.ascontiguousarray(x.astype(f).T))  # [D, B]
    xts = [np.ascontiguousarray(xT[:, c * BS:(c + 1) * BS]) for c in range(N_CORES)]
    return shared, xts


def run(inputs, trace=False):
    nc = build_bass()
    shared, xts = prep_inputs(**inputs)
    in_maps = [dict(shared, xt=xts[c]) for c in range(N_CORES)]
    res = run_bass_kernel_spmd(
        nc, in_maps, core_ids=list(range(N_CORES)), trace=trace
    )
    y = np.concatenate([r["y"] for r in res.results], axis=0)
    return y, res


def kernel(**inputs):
    y, _ = run(inputs, trace=False)
    return y


if __name__ == "__main__":
    rng = np.random.default_rng(0)
    ins = {
        "x": rng.standard_normal((B, D), dtype=np.float32),
        "W1": rng.standard_normal((E, D, H), dtype=np.float32) / 32,
        "b1": rng.standard_normal((E, H), dtype=np.float32) / 32,
        "W2": rng.standard_normal((E, H), dtype=np.float32) / 32,
        "b2": rng.standard_normal((E,), dtype=np.float32) / 32,
        "Wg": rng.standard_normal((D, E), dtype=np.float32) / 32,
        "bg": rng.standard_normal((E,), dtype=np.float32) / 32,
    }
    y = kernel(**ins)
    print("ok", y.shape, y.dtype)



# revision 8
# speedup vs baseline: 1.2893x; 1.2893x over previous
"""MoE kernel for TRN2, 8 NeuronCores, data-parallel over the batch dim.

Reference computation (B=8192, D=1024, H=1024, E=16):
    weights = softmax(x @ Wg + bg, axis=1)            # [B, E]
    h       = relu(einsum('bd,edh->beh', x, W1) + b1) # [B, E, H]
    eo      = einsum('beh,eh->be', h, W2) + b2        # [B, E]
    out     = sum(eo * weights, axis=1, keepdims=True)# [B, 1]

Strategy (v2):
  - Shard B over 8 cores (1024 rows/core); weights replicated. bf16 inputs
    (rel err ~2.6e-3, well within tolerance).
  - x-stationary matmul: stationary = xT tile [128d, 128b] (reused by 4
    consecutive N=512 matmuls, so LDWEIGHTS latency is fully hidden);
    moving = W1f chunks where W1f[d,(e,h)] = W1[e,d,h]*W2[e,h] (W2 folded
    on host). psum out = [128b, 2048] covering 2 experts' h columns.
  - Stage 2 never touches the PE: relu(z+b1)*W2 == max(z*W2, -b1*W2) when
    W2>0 and min(z*W2, -b1*W2) when W2<0. Columns are sign-sorted per
    expert on host, so each expert needs one max-range and one min-range
    DVE tensor_tensor_reduce (fused elementwise+reduce, init-chained),
    reading psum directly. c2[e] = b2[e] + sum_h b1*W2 folds the constants.
  - Gating: same as v1 (logits [128b,16e] psum, softmax along free dim,
    exp(bg) folded multiplicatively), but bf16 operands.
  - Final combine: one tensor_tensor_reduce (eo*gates, reduce) per b-tile.
"""

import numpy as np
import ml_dtypes

import concourse.bacc as bacc
import concourse.bass as bass
import concourse.mybir as mybir
from concourse import tile
from concourse.bass_utils import run_bass_kernel_spmd

B, D, H, E = 8192, 1024, 1024, 16
N_CORES = 8
BS = B // N_CORES  # 1024 batch rows per core
NB = BS // 128     # 8 b-tiles of 128
DT = D // 128      # 8 d-tiles
NC = E // 2        # 8 chunks of 2 experts (2048 h-columns each)
CW = 2 * H         # 2048 chunk width

F32 = mybir.dt.float32
BF16 = mybir.dt.bfloat16
AF = mybir.ActivationFunctionType
AX = mybir.AxisListType
OP = mybir.AluOpType
NPBF16 = ml_dtypes.bfloat16


def build_bass(pcount):
    """pcount[e] = number of positive-W2 columns for expert e (host-known)."""
    nc = bacc.Bacc("TRN2", target_bir_lowering=False, debug=False)
    xt_d = nc.dram_tensor("xt", [D, BS], BF16, kind="ExternalInput")
    w1f_d = nc.dram_tensor("w1f", [NC, DT, 128, CW], BF16, kind="ExternalInput")
    ttab_d = nc.dram_tensor("ttab", [NC, 128, CW], BF16, kind="ExternalInput")
    wgp_d = nc.dram_tensor("wgp", [128, DT, E], BF16, kind="ExternalInput")
    ebg_d = nc.dram_tensor("ebg", [128, E], F32, kind="ExternalInput")
    c2_d = nc.dram_tensor("c2", [128, E], F32, kind="ExternalInput")
    y_d = nc.dram_tensor("y", [BS, 1], F32, kind="ExternalOutput")

    with tile.TileContext(nc) as tc:
        with (
            tc.tile_pool(name="const", bufs=1) as cpool,
            tc.tile_pool(name="w1", bufs=2) as w1pool,
            tc.tile_pool(name="scr", bufs=2) as scrpool,
            tc.tile_pool(name="sm", bufs=2) as smpool,
            tc.tile_pool(name="ps", bufs=2, space=bass.MemorySpace.PSUM) as psp,
        ):
            # ---- resident tensors ----
            xt_sb = []
            for d in range(DT):
                tl = cpool.tile([128, BS], BF16, tag=f"xt{d}")
                nc.sync.dma_start(tl[:], xt_d[d * 128:(d + 1) * 128, :])
                xt_sb.append(tl)
            wgp_sb = cpool.tile([128, DT, E], BF16, tag="wgp")
            nc.sync.dma_start(wgp_sb[:], wgp_d[:])
            ebg_sb = cpool.tile([128, E], F32, tag="ebg")
            nc.sync.dma_start(ebg_sb[:], ebg_d[:])
            c2_sb = cpool.tile([128, E], F32, tag="c2")
            nc.sync.dma_start(c2_sb[:], c2_d[:])
            ttab_sb = cpool.tile([128, NC, CW], BF16, tag="ttab")
            w_all = cpool.tile([128, NB, E], F32, tag="wall")  # gate weights
            eo_sb = cpool.tile([128, NB, E], F32, tag="eo")    # expert outputs

            # ---- gating: logits -> softmax along free dim ----
            for bt in range(NB):
                ps_g = psp.tile([128, CW], F32, tag="ps")
                for d in range(DT):
                    nc.tensor.matmul(
                        ps_g[:, :E],
                        xt_sb[d][:, bt * 128:(bt + 1) * 128],
                        wgp_sb[:, d, :],
                        start=(d == 0), stop=(d == DT - 1),
                        skip_group_check=True,
                    )
                pexp = smpool.tile([128, E], F32, tag="pexp")
                nc.scalar.activation(pexp[:], ps_g[:, :E], AF.Exp)
                nc.vector.tensor_mul(pexp[:], pexp[:], ebg_sb[:])
                ssum = smpool.tile([128, 1], F32, tag="ssum")
                nc.vector.reduce_sum(ssum[:], pexp[:], axis=AX.X)
                rsum = smpool.tile([128, 1], F32, tag="rsum")
                nc.vector.reciprocal(rsum[:], ssum[:])
                nc.vector.tensor_scalar_mul(w_all[:, bt, :], pexp[:], rsum[:])

            # ---- main loop: chunks of 2 experts ----
            for c in range(NC):
                w1t = w1pool.tile([128, DT, CW], BF16, tag="w1t")
                for d in range(DT):
                    nc.sync.dma_start(w1t[:, d, :], w1f_d[c, d, :, :])
                nc.sync.dma_start(ttab_sb[:, c, :], ttab_d[c, :, :])
                for bt in range(NB):
                    ps1 = psp.tile([128, CW], F32, tag="ps")
                    for d in range(DT):
                        lhs = xt_sb[d][:, bt * 128:(bt + 1) * 128]
                        for n in range(4):
                            nc.tensor.matmul(
                                ps1[:, n * 512:(n + 1) * 512],
                                lhs,
                                w1t[:, d, n * 512:(n + 1) * 512],
                                start=(d == 0), stop=(d == DT - 1),
                                skip_group_check=True,
                            )
                    scr = scrpool.tile([128, CW], BF16, tag="scr")
                    for j in range(2):
                        e = 2 * c + j
                        base = j * H
                        p = int(pcount[e])
                        nc.vector.tensor_tensor(
                            scr[:, base:base + p],
                            ps1[:, base:base + p],
                            ttab_sb[:, c, base:base + p],
                            op=OP.max,
                        )
                        nc.vector.tensor_tensor(
                            scr[:, base + p:base + H],
                            ps1[:, base + p:base + H],
                            ttab_sb[:, c, base + p:base + H],
                            op=OP.min,
                        )
                    for j in range(2):
                        e = 2 * c + j
                        nc.vector.reduce_sum(
                            eo_sb[:, bt, e:e + 1],
                            scr[:, j * H:(j + 1) * H],
                            axis=AX.X,
                        )

            # ---- combine: out[b] = sum_e gate * (eo + c2) ----
            for bt in range(NB):
                eo2 = smpool.tile([128, E], F32, tag="eo2")
                nc.vector.tensor_tensor(
                    eo2[:], eo_sb[:, bt, :], c2_sb[:], op=OP.add)
                prod = smpool.tile([128, E], F32, tag="prod")
                nc.vector.tensor_tensor(
                    prod[:], eo2[:], w_all[:, bt, :], op=OP.mult)
                y_t = smpool.tile([128, 1], F32, tag="yt")
                nc.vector.reduce_sum(y_t[:], prod[:], axis=AX.X)
                nc.sync.dma_start(y_d[bt * 128:(bt + 1) * 128, :], y_t[:])
    nc.compile()
    return nc


def prep_inputs(x, W1, b1, W2, b2, Wg, bg):
    """Host-side data prep. Returns (shared_map, per_core_xt, pcount)."""
    f = np.float32
    W1 = np.asarray(W1, f)
    b1 = np.asarray(b1, f)
    W2 = np.asarray(W2, f)
    # sign-sort columns per expert: positive W2 first
    perm = np.argsort(W2 <= 0, axis=1, kind="stable")  # [E, H]
    pcount = (W2 > 0).sum(axis=1)                      # [E]
    W1p = np.take_along_axis(W1 * W2[:, None, :], perm[:, None, :], axis=2)
    thr = np.take_along_axis(-b1 * W2, perm, axis=1)   # [E, H]
    # W1p [E, D, H] -> w1f [NC, DT, 128, (2, H)]
    w1f = np.ascontiguousarray(
        W1p.reshape(NC, 2, DT, 128, H).transpose(0, 2, 3, 1, 4)
        .reshape(NC, DT, 128, CW).astype(NPBF16))
    ttab = np.ascontiguousarray(np.broadcast_to(
        thr.reshape(NC, 1, CW), (NC, 128, CW)).astype(NPBF16))
    c2 = b2.astype(f) + (b1 * W2).sum(axis=1)
    c2 = np.ascontiguousarray(np.broadcast_to(c2[None, :], (128, E)).astype(f))
    wgp = np.ascontiguousarray(
        Wg.reshape(DT, 128, E).transpose(1, 0, 2).astype(NPBF16))
    ebg = np.ascontiguousarray(np.broadcast_to(
        np.exp(bg.astype(f))[None, :], (128, E)).astype(f))
    shared = {"w1f": w1f, "ttab": ttab, "wgp": wgp, "ebg": ebg, "c2": c2}
    xT = np.ascontiguousarray(np.asarray(x, f).T.astype(NPBF16))  # [D, B]
    xts = [np.ascontiguousarray(xT[:, c * BS:(c + 1) * BS]) for c in range(N_CORES)]
    return shared, xts, pcount


def run(inputs, trace=False):
    shared, xts, pcount = prep_inputs(**inputs)
    nc = build_bass(pcount)
    in_maps = [dict(shared, xt=xts[c]) for c in range(N_CORES)]
    res = run_bass_kernel_spmd(
        nc, in_maps, core_ids=list(range(N_CORES)), trace=trace
    )
    y = np.concatenate([r["y"] for r in res.results], axis=0)
    return y, res


def kernel(**inputs):
    y, _ = run(inputs, trace=False)
    return y


if __name__ == "__main__":
    rng = np.random.default_rng(0)
    ins = {
        "x": rng.standard_normal((B, D), dtype=np.float32),
        "W1": rng.standard_normal((E, D, H), dtype=np.float32) / 32,
        "b1": rng.standard_normal((E, H), dtype=np.float32) / 32,
        "W2": rng.standard_normal((E, H), dtype=np.float32) / 32,
        "b2": rng.standard_normal((E,), dtype=np.float32) / 32,
        "Wg": rng.standard_normal((D, E), dtype=np.float32) / 32,
        "bg": rng.standard_normal((E,), dtype=np.float32) / 32,
    }
    y = kernel(**ins)
    print("ok", y.shape, y.dtype)


# revision 10
# speedup vs baseline: 1.2930x; 1.0029x over previous
"""MoE kernel for TRN2, 8 NeuronCores, data-parallel over the batch dim.

Reference computation (B=8192, D=1024, H=1024, E=16):
    weights = softmax(x @ Wg + bg, axis=1)            # [B, E]
    h       = relu(einsum('bd,edh->beh', x, W1) + b1) # [B, E, H]
    eo      = einsum('beh,eh->be', h, W2) + b2        # [B, E]
    out     = sum(eo * weights, axis=1, keepdims=True)# [B, 1]

Strategy (v4):
  - Shard B over 8 cores (1024 rows/core); weights replicated. bf16 inputs
    (rel err ~3e-3, well within tolerance).
  - x-stationary matmul: stationary = xT tile [128d, 128b] (reused by the
    2-3 consecutive N=512 matmuls of each d step, so LDWEIGHTS latency is
    fully hidden); moving = W1f chunks where W1f[d,(e,h)] = W1[e,d,h]*W2[e,h]
    (W2 folded on host). One expert per chunk: psum out = [128b, 1024h].
  - Stage 2 never touches the PE: relu(z+b1)*W2 == max(z*W2, -b1*W2) when
    W2>0 and min(z*W2, -b1*W2) when W2<0. Columns are sign-sorted per
    expert on host, so each expert needs one max-range and one min-range
    scalar_tensor_tensor (DVE; fused elementwise + free-dim sum via
    accum_out), reading psum directly. The two partial sums are added on
    GpSimd. c2[e] = b2[e] + sum_h b1*W2 is added in the final combine.
  - Gating is interleaved into chunk 0's d-loop (shares the stationary
    LDW): logits [128b, 16e] in a dedicated psum pool, softmax along the
    free dim with exp(bg) folded multiplicatively.
  - Final combine per b-tile: (eo + c2) * gates on DVE, reduce, DMA out.
"""

import numpy as np
import ml_dtypes

import concourse.bacc as bacc
import concourse.bass as bass
import concourse.mybir as mybir
from concourse import tile
from concourse.bass_utils import run_bass_kernel_spmd

B, D, H, E = 8192, 1024, 1024, 16
N_CORES = 8
BS = B // N_CORES  # 1024 batch rows per core
NB = BS // 128     # 8 b-tiles of 128
DT = D // 128      # 8 d-tiles
NC = E             # 16 chunks, one expert each
CW = H             # 1024 chunk width

F32 = mybir.dt.float32
BF16 = mybir.dt.bfloat16
AF = mybir.ActivationFunctionType
AX = mybir.AxisListType
OP = mybir.AluOpType
NPBF16 = ml_dtypes.bfloat16


def build_bass(pcount):
    """pcount[e] = number of positive-W2 columns for expert e (host-known)."""
    nc = bacc.Bacc("TRN2", target_bir_lowering=False, debug=False)
    xt_d = nc.dram_tensor("xt", [D, BS], BF16, kind="ExternalInput")
    w1f_d = nc.dram_tensor("w1f", [NC, DT, 128, CW], BF16, kind="ExternalInput")
    ttab_d = nc.dram_tensor("ttab", [NC, 128, CW], BF16, kind="ExternalInput")
    wgp_d = nc.dram_tensor("wgp", [128, DT, E], BF16, kind="ExternalInput")
    ebg_d = nc.dram_tensor("ebg", [128, E], F32, kind="ExternalInput")
    c2_d = nc.dram_tensor("c2", [128, E], F32, kind="ExternalInput")
    y_d = nc.dram_tensor("y", [BS, 1], F32, kind="ExternalOutput")

    with tile.TileContext(nc) as tc:
        with (
            tc.tile_pool(name="const", bufs=1) as cpool,
            tc.tile_pool(name="w1", bufs=2) as w1pool,
            tc.tile_pool(name="scr", bufs=3) as scrpool,
            tc.tile_pool(name="sm", bufs=2) as smpool,
            tc.tile_pool(name="ps", bufs=3, space=bass.MemorySpace.PSUM) as psp,
            tc.tile_pool(name="psg", bufs=2, space=bass.MemorySpace.PSUM) as psg,
        ):
            # ---- resident tensors ----
            xt_sb = []
            for d in range(DT):
                tl = cpool.tile([128, BS], BF16, tag=f"xt{d}")
                nc.sync.dma_start(tl[:], xt_d[d * 128:(d + 1) * 128, :])
                xt_sb.append(tl)
            wgp_sb = cpool.tile([128, DT, E], BF16, tag="wgp")
            nc.sync.dma_start(wgp_sb[:], wgp_d[:])
            ebg_sb = cpool.tile([128, E], F32, tag="ebg")
            nc.sync.dma_start(ebg_sb[:], ebg_d[:])
            c2_sb = cpool.tile([128, E], F32, tag="c2")
            nc.sync.dma_start(c2_sb[:], c2_d[:])
            ttab_sb = cpool.tile([128, NC, CW], BF16, tag="ttab")
            w_all = cpool.tile([128, NB, E], F32, tag="wall")  # gate weights
            eo_sb = cpool.tile([128, NB, E], F32, tag="eo")    # expert outputs

            # ---- main loop: one expert per chunk ----
            for c in range(NC):
                w1t = w1pool.tile([128, DT, CW], BF16, tag="w1t")
                for d in range(DT):
                    nc.sync.dma_start(w1t[:, d, :], w1f_d[c, d, :, :])
                nc.sync.dma_start(ttab_sb[:, c, :], ttab_d[c, :, :])
                for bt in range(NB):
                    ps1 = psp.tile([128, CW], F32, tag="ps")
                    ps_g = None
                    if c == 0:
                        ps_g = psg.tile([128, E], F32, name="ps_g", tag="psg")
                    for d in range(DT):
                        lhs = xt_sb[d][:, bt * 128:(bt + 1) * 128]
                        for n in range(2):
                            nc.tensor.matmul(
                                ps1[:, n * 512:(n + 1) * 512],
                                lhs,
                                w1t[:, d, n * 512:(n + 1) * 512],
                                start=(d == 0), stop=(d == DT - 1),
                                skip_group_check=True,
                            )
                        if c == 0:
                            nc.tensor.matmul(
                                ps_g[:],
                                lhs,
                                wgp_sb[:, d, :],
                                start=(d == 0), stop=(d == DT - 1),
                                skip_group_check=True,
                            )
                    if c == 0:
                        # softmax along free dim; exp(bg) folded in
                        pexp = smpool.tile([128, E], F32, tag="pexp")
                        nc.scalar.activation(pexp[:], ps_g[:], AF.Exp)
                        nc.vector.tensor_mul(pexp[:], pexp[:], ebg_sb[:])
                        ssum = smpool.tile([128, 1], F32, tag="ssum")
                        nc.vector.reduce_sum(ssum[:], pexp[:], axis=AX.X)
                        rsum = smpool.tile([128, 1], F32, tag="rsum")
                        nc.vector.reciprocal(rsum[:], ssum[:])
                        nc.vector.tensor_scalar_mul(w_all[:, bt, :], pexp[:], rsum[:])
                    # fused max/min + free-dim sum, psum -> acc pair
                    p = int(pcount[c])
                    scr = scrpool.tile([128, CW], BF16, tag="scr")
                    acc = scrpool.tile([128, 2], F32, tag="acc")
                    nc.vector.scalar_tensor_tensor(
                        scr[:, 0:p], ps1[:, 0:p], 0.0, ttab_sb[:, c, 0:p],
                        OP.bypass, OP.max, accum_out=acc[:, 0:1])
                    nc.vector.scalar_tensor_tensor(
                        scr[:, p:CW], ps1[:, p:CW], 0.0, ttab_sb[:, c, p:CW],
                        OP.bypass, OP.min, accum_out=acc[:, 1:2])
                    nc.gpsimd.tensor_tensor(
                        eo_sb[:, bt, c:c + 1], acc[:, 0:1], acc[:, 1:2], op=OP.add)

            # ---- combine: out[b] = sum_e gate * (eo + c2) ----
            for bt in range(NB):
                eo2 = smpool.tile([128, E], F32, tag="eo2")
                nc.vector.tensor_tensor(
                    eo2[:], eo_sb[:, bt, :], c2_sb[:], op=OP.add)
                prod = smpool.tile([128, E], F32, tag="prod")
                nc.vector.tensor_tensor(
                    prod[:], eo2[:], w_all[:, bt, :], op=OP.mult)
                y_t = smpool.tile([128, 1], F32, tag="yt")
                nc.vector.reduce_sum(y_t[:], prod[:], axis=AX.X)
                nc.sync.dma_start(y_d[bt * 128:(bt + 1) * 128, :], y_t[:])
    nc.compile()
    return nc


def prep_inputs(x, W1, b1, W2, b2, Wg, bg):
    """Host-side data prep. Returns (shared_map, per_core_xt, pcount)."""
    f = np.float32
    W1 = np.asarray(W1, f)
    b1 = np.asarray(b1, f)
    W2 = np.asarray(W2, f)
    # sign-sort columns per expert: positive W2 first
    perm = np.argsort(W2 <= 0, axis=1, kind="stable")  # [E, H]
    pcount = (W2 > 0).sum(axis=1)                      # [E]
    W1p = np.take_along_axis(W1 * W2[:, None, :], perm[:, None, :], axis=2)
    thr = np.take_along_axis(-b1 * W2, perm, axis=1)   # [E, H]
    # W1p [E, D, H] -> w1f [NC=E, DT, 128, H]
    w1f = np.ascontiguousarray(
        W1p.reshape(E, DT, 128, H).astype(NPBF16))
    ttab = np.ascontiguousarray(np.broadcast_to(
        thr.reshape(NC, 1, CW), (NC, 128, CW)).astype(NPBF16))
    c2 = b2.astype(f) + (b1 * W2).sum(axis=1)
    c2 = np.ascontiguousarray(np.broadcast_to(c2[None, :], (128, E)).astype(f))
    wgp = np.ascontiguousarray(
        Wg.reshape(DT, 128, E).transpose(1, 0, 2).astype(NPBF16))
    ebg = np.ascontiguousarray(np.broadcast_to(
        np.exp(bg.astype(f))[None, :], (128, E)).astype(f))
    shared = {"w1f": w1f, "ttab": ttab, "wgp": wgp, "ebg": ebg, "c2": c2}
    xT = np.ascontiguousarray(np.asarray(x, f).T.astype(NPBF16))  # [D, B]
    xts = [np.ascontiguousarray(xT[:, c * BS:(c + 1) * BS]) for c in range(N_CORES)]
    return shared, xts, pcount


def run(inputs, trace=False):
    shared, xts, pcount = prep_inputs(**inputs)
    nc = build_bass(pcount)
    in_maps = [dict(shared, xt=xts[c]) for c in range(N_CORES)]
    res = run_bass_kernel_spmd(
        nc, in_maps, core_ids=list(range(N_CORES)), trace=trace
    )
    y = np.concatenate([r["y"] for r in res.results], axis=0)
    return y, res


def kernel(**inputs):
    y, _ = run(inputs, trace=False)
    return y


if __name__ == "__main__":
    rng = np.random.default_rng(0)
    ins = {
        "x": rng.standard_normal((B, D), dtype=np.float32),
        "W1": rng.standard_normal((E, D, H), dtype=np.float32) / 32,
        "b1": rng.standard_normal((E, H), dtype=np.float32) / 32,
        "W2": rng.standard_normal((E, H), dtype=np.float32) / 32,
        "b2": rng.standard_normal((E,), dtype=np.float32) / 32,
        "Wg": rng.standard_normal((D, E), dtype=np.float32) / 32,
        "bg": rng.standard_normal((E,), dtype=np.float32) / 32,
    }
    y = kernel(**ins)
    print("ok", y.shape, y.dtype)


# revision 15
# speedup vs baseline: 1.4848x; 1.1483x over previous
"""MoE kernel for TRN2, 8 NeuronCores, data-parallel over the batch dim.

Reference computation (B=8192, D=1024, H=1024, E=16):
    weights = softmax(x @ Wg + bg, axis=1)            # [B, E]
    h       = relu(einsum('bd,edh->beh', x, W1) + b1) # [B, E, H]
    eo      = einsum('beh,eh->be', h, W2) + b2        # [B, E]
    out     = sum(eo * weights, axis=1, keepdims=True)# [B, 1]

Strategy (v5):
  - Shard B over 8 cores (1024 rows/core); weights replicated.
  - x-stationary matmul: stationary = xT tile [128d, 128b] (reused by the
    consecutive N=512 matmuls of each d step, so LDWEIGHTS latency is fully
    hidden); moving = W1f chunks where W1f[d,(e,h)] = W1[e,d,h]*W2[e,h]
    (W2 folded on host). One expert per chunk: psum out = [128b, 1024h].
  - Mixed precision stage 1: d-tiles 0-5 in bf16, d-tiles 6-7 as ONE fp8
    DoubleRow matmul group (2 fp8 weights/cell, ~1.8x the bf16 rate here
    since LDWEIGHTS is amortized). Scale bookkeeping: the whole stage-1 is
    scaled by S=2^15 (bf16 weights xS; fp8 pair: x*16, W1f*2048 so the
    product is also xS), thresholds xS, and the final pair-sum is
    multiplied by 1/S. Measured end-to-end rel err ~1.85e-2 (gate 2e-2,
    deterministic inputs).
  - Stage 2 never touches the PE: relu(z+b1)*W2 == max(z*W2, -b1*W2) when
    W2>0 and min(z*W2, -b1*W2) when W2<0. Columns are sign-sorted per
    expert on host, so each expert needs one max-range and one min-range
    scalar_tensor_tensor (DVE; fused elementwise + free-dim sum via
    accum_out), reading psum directly. GpSimd computes (acc0+acc1)/S.
    c2[e] = b2[e] + sum_h b1*W2 is added in the final combine.
  - Gating is interleaved into chunk 0's d-loop (shares the stationary
    LDW): logits [128b, 16e] psum, softmax along the free dim with
    exp(bg) folded multiplicatively. Gating runs on the unscaled bf16 xT.
  - Final combine per b-tile: (eo + c2) * gates on DVE, reduce into a
    [128, NB] staging tile, single DMA out at the end.
"""

import numpy as np
import ml_dtypes

import concourse.bacc as bacc
import concourse.bass as bass
import concourse.mybir as mybir
from concourse import tile
from concourse.bass_utils import run_bass_kernel_spmd

B, D, H, E = 8192, 1024, 1024, 16
N_CORES = 8
BS = B // N_CORES  # 1024 batch rows per core
NB = BS // 128     # 8 b-tiles of 128
DT = D // 128      # 8 d-tiles total
DB = 6             # d-tiles 0-5: bf16
NC = E             # 16 chunks, one expert each
CW = H             # 1024 chunk width
S = 32768.0        # stage-1 scale (bf16 weights xS; fp8: 16 * 2048)

F32 = mybir.dt.float32
BF16 = mybir.dt.bfloat16
FP8 = mybir.dt.float8e4
AF = mybir.ActivationFunctionType
AX = mybir.AxisListType
OP = mybir.AluOpType
PM = mybir.MatmulPerfMode
NPBF16 = ml_dtypes.bfloat16
NPFP8 = mybir.dt.np(FP8)


def build_bass(pcount):
    """pcount[e] = number of positive-W2 columns for expert e (host-known)."""
    nc = bacc.Bacc("TRN2", target_bir_lowering=False, debug=False)
    xt_d = nc.dram_tensor("xt", [D, BS], BF16, kind="ExternalInput")
    xp8_d = nc.dram_tensor("xp8", [128, 2, BS], FP8, kind="ExternalInput")
    w1b_d = nc.dram_tensor("w1b", [NC, DB, 128, CW], BF16, kind="ExternalInput")
    w1p8_d = nc.dram_tensor("w1p8", [NC, 128, 2, CW], FP8, kind="ExternalInput")
    ttab_d = nc.dram_tensor("ttab", [NC, 128, CW], BF16, kind="ExternalInput")
    wgp_d = nc.dram_tensor("wgp", [128, DT, E], BF16, kind="ExternalInput")
    ebg_d = nc.dram_tensor("ebg", [128, E], F32, kind="ExternalInput")
    c2_d = nc.dram_tensor("c2", [128, E], F32, kind="ExternalInput")
    y_d = nc.dram_tensor("y", [128, NB], F32, kind="ExternalOutput")

    with tile.TileContext(nc) as tc:
        with (
            tc.tile_pool(name="const", bufs=1) as cpool,
            tc.tile_pool(name="wstream", bufs=2) as wpool,
            tc.tile_pool(name="work", bufs=3) as wk,
            tc.tile_pool(name="ps", bufs=3, space=bass.MemorySpace.PSUM) as psp,
            tc.tile_pool(name="psg", bufs=2, space=bass.MemorySpace.PSUM) as psgp,
        ):
            # ---- resident tensors ----
            xt_sb = []
            for d in range(DT):
                tl = cpool.tile([128, BS], BF16, tag=f"xt{d}")
                nc.sync.dma_start(tl[:], xt_d[d * 128:(d + 1) * 128, :])
                xt_sb.append(tl)
            xp8_sb = cpool.tile([128, 2, BS], FP8, tag="xp8")
            nc.sync.dma_start(xp8_sb[:], xp8_d[:])
            wgp_sb = cpool.tile([128, DT, E], BF16, tag="wgp")
            nc.sync.dma_start(wgp_sb[:], wgp_d[:])
            ebg_sb = cpool.tile([128, E], F32, tag="ebg")
            nc.sync.dma_start(ebg_sb[:], ebg_d[:])
            c2_sb = cpool.tile([128, E], F32, tag="c2")
            nc.sync.dma_start(c2_sb[:], c2_d[:])
            ttab_sb = cpool.tile([128, NC, CW], BF16, tag="ttab")
            inv_sb = cpool.tile([128, 1], F32, tag="inv")
            nc.vector.memset(inv_sb[:], 1.0 / S)
            w_all = cpool.tile([128, NB, E], F32, tag="wall")  # gate weights
            eo_sb = cpool.tile([128, NB, E], F32, tag="eo")    # expert outputs
            yall = cpool.tile([128, NB], F32, tag="yall")

            # ---- main loop: one expert per chunk ----
            for c in range(NC):
                w1t = wpool.tile([128, DB, CW], BF16, tag="w1t")
                for d in range(DB):
                    nc.sync.dma_start(w1t[:, d, :], w1b_d[c, d, :, :])
                w1p8t = wpool.tile([128, 2, CW], FP8, tag="w1p8t")
                nc.sync.dma_start(w1p8t[:], w1p8_d[c, :, :, :])
                nc.sync.dma_start(ttab_sb[:, c, :], ttab_d[c, :, :])
                for bt in range(NB):
                    ps1 = psp.tile([128, CW], F32, tag="ps")
                    ps_g = None
                    if c == 0:
                        ps_g = psgp.tile([128, E], F32, name="ps_g", tag="psg")
                    for d in range(DB):
                        lhs = xt_sb[d][:, bt * 128:(bt + 1) * 128]
                        for n in range(2):
                            nc.tensor.matmul(
                                ps1[:, n * 512:(n + 1) * 512],
                                lhs,
                                w1t[:, d, n * 512:(n + 1) * 512],
                                start=(d == 0), stop=False,
                                skip_group_check=True,
                            )
                        if c == 0:
                            nc.tensor.matmul(
                                ps_g[:],
                                lhs,
                                wgp_sb[:, d, :],
                                start=(d == 0), stop=False,
                                skip_group_check=True,
                            )
                    # fp8 DoubleRow pair covers d-tiles 6 and 7
                    for n in range(4):
                        nc.tensor.matmul(
                            ps1[:, n * 256:(n + 1) * 256],
                            xp8_sb[:, :, bt * 128:(bt + 1) * 128],
                            w1p8t[:, :, n * 256:(n + 1) * 256],
                            start=False, stop=True,
                            perf_mode=PM.DoubleRow,
                            skip_group_check=True,
                        )
                    if c == 0:
                        # finish the gating group on the bf16 xT tiles
                        for d in range(DB, DT):
                            nc.tensor.matmul(
                                ps_g[:],
                                xt_sb[d][:, bt * 128:(bt + 1) * 128],
                                wgp_sb[:, d, :],
                                start=False, stop=(d == DT - 1),
                                skip_group_check=True,
                            )
                        # softmax along free dim; exp(bg) folded in
                        pexp = wk.tile([128, E], F32, tag="pexp")
                        nc.scalar.activation(pexp[:], ps_g[:], AF.Exp)
                        nc.vector.tensor_mul(pexp[:], pexp[:], ebg_sb[:])
                        ssum = wk.tile([128, 1], F32, tag="ssum")
                        nc.vector.reduce_sum(ssum[:], pexp[:], axis=AX.X)
                        rsum = wk.tile([128, 1], F32, tag="rsum")
                        nc.vector.reciprocal(rsum[:], ssum[:])
                        nc.vector.tensor_scalar_mul(w_all[:, bt, :], pexp[:], rsum[:])
                    # fused max/min + free-dim sum, psum -> acc pair
                    p = int(pcount[c])
                    scr = wk.tile([128, CW], BF16, tag="scr")
                    acc = wk.tile([128, 2], F32, tag="acc")
                    nc.vector.scalar_tensor_tensor(
                        scr[:, 0:p], ps1[:, 0:p], 0.0, ttab_sb[:, c, 0:p],
                        OP.bypass, OP.max, accum_out=acc[:, 0:1])
                    nc.vector.scalar_tensor_tensor(
                        scr[:, p:CW], ps1[:, p:CW], 0.0, ttab_sb[:, c, p:CW],
                        OP.bypass, OP.min, accum_out=acc[:, 1:2])
                    # eo = acc0 + acc1 (still scaled by S)
                    nc.gpsimd.tensor_tensor(
                        eo_sb[:, bt, c:c + 1], acc[:, 0:1], acc[:, 1:2],
                        op=OP.add)

            # ---- combine: out[b] = sum_e gate * (eo/S + c2) ----
            for bt in range(NB):
                eo2 = wk.tile([128, E], F32, tag="eo2")
                nc.vector.scalar_tensor_tensor(
                    eo2[:], eo_sb[:, bt, :], inv_sb[:], c2_sb[:],
                    OP.mult, OP.add)
                prod = wk.tile([128, E], F32, tag="prod")
                nc.vector.tensor_tensor(
                    prod[:], eo2[:], w_all[:, bt, :], op=OP.mult)
                nc.vector.reduce_sum(yall[:, bt:bt + 1], prod[:], axis=AX.X)
            nc.sync.dma_start(y_d[:], yall[:])
    nc.compile()
    return nc


def q8(a):
    return np.clip(a, -240, 240).astype(np.float32).astype(NPFP8)


def prep_inputs(x, W1, b1, W2, b2, Wg, bg):
    """Host-side data prep. Returns (shared_map, per-core lists, pcount)."""
    f = np.float32
    W1 = np.asarray(W1, f)
    b1 = np.asarray(b1, f)
    W2 = np.asarray(W2, f)
    # sign-sort columns per expert: positive W2 first
    perm = np.argsort(W2 <= 0, axis=1, kind="stable")  # [E, H]
    pcount = (W2 > 0).sum(axis=1)                      # [E]
    W1p = np.take_along_axis(W1 * W2[:, None, :], perm[:, None, :], axis=2)
    thr = np.take_along_axis(-b1 * W2, perm, axis=1)   # [E, H]
    # bf16 part: d rows 0:768, scaled by S
    w1b = np.ascontiguousarray(
        (W1p[:, :DB * 128, :] * S).reshape(E, DB, 128, CW).astype(NPBF16))
    # fp8 DoubleRow pair: d rows 768:1024, scaled by 2048 -> [NC, 128, 2, CW]
    w1p8 = np.ascontiguousarray(
        q8(W1p[:, DB * 128:, :] * 2048.0).reshape(E, 2, 128, CW)
        .transpose(0, 2, 1, 3))
    ttab = np.ascontiguousarray(np.broadcast_to(
        (thr * S).reshape(NC, 1, CW), (NC, 128, CW)).astype(NPBF16))
    c2 = b2.astype(f) + (b1 * W2).sum(axis=1)
    c2 = np.ascontiguousarray(np.broadcast_to(c2[None, :], (128, E)).astype(f))
    wgp = np.ascontiguousarray(
        Wg.reshape(DT, 128, E).transpose(1, 0, 2).astype(NPBF16))
    ebg = np.ascontiguousarray(np.broadcast_to(
        np.exp(bg.astype(f))[None, :], (128, E)).astype(f))
    shared = {"w1b": w1b, "w1p8": w1p8, "ttab": ttab, "wgp": wgp,
              "ebg": ebg, "c2": c2}
    xT = np.ascontiguousarray(np.asarray(x, f).T)  # [D, B]
    xtb = xT.astype(NPBF16)
    xq8 = q8(xT[DB * 128:] * 16.0).reshape(2, 128, B).transpose(1, 0, 2)
    xts = [np.ascontiguousarray(xtb[:, c * BS:(c + 1) * BS])
           for c in range(N_CORES)]
    x8s = [np.ascontiguousarray(xq8[:, :, c * BS:(c + 1) * BS])
           for c in range(N_CORES)]
    return shared, xts, x8s, pcount


def run(inputs, trace=False):
    shared, xts, x8s, pcount = prep_inputs(**inputs)
    nc = build_bass(pcount)
    in_maps = [dict(shared, xt=xts[c], xp8=x8s[c]) for c in range(N_CORES)]
    res = run_bass_kernel_spmd(
        nc, in_maps, core_ids=list(range(N_CORES)), trace=trace
    )
    # y comes back [128, NB] per core; b = bt*128 + p
    y = np.concatenate(
        [np.asarray(r["y"]).T.reshape(BS, 1) for r in res.results], axis=0)
    return y, res


def kernel(**inputs):
    y, _ = run(inputs, trace=False)
    return y


if __name__ == "__main__":
    rng = np.random.default_rng(0)
    ins = {
        "x": rng.standard_normal((B, D), dtype=np.float32),
        "W1": rng.standard_normal((E, D, H), dtype=np.float32) / 32,
        "b1": rng.standard_normal((E, H), dtype=np.float32) / 32,
        "W2": rng.standard_normal((E, H), dtype=np.float32) / 32,
        "b2": rng.standard_normal((E,), dtype=np.float32) / 32,
        "Wg": rng.standard_normal((D, E), dtype=np.float32) / 32,
        "bg": rng.standard_normal((E,), dtype=np.float32) / 32,
    }
    y = kernel(**ins)
    print("ok", y.shape, y.dtype)
